# revision 9
# baseline (speedup 1.0000x reference)
"""AttentionPooling Trainium2 kernel (8 NeuronCores, SPMD).

Reference computation:
    scores = tanh(x @ W1 + b1) @ W2          # [N, 4]
    w      = segment_softmax(scores, batch)  # per-graph softmax over nodes
    out[g] = mean_h( sum_{n in g} w[n,h] * x[n] )   # [G, 256]

Sharding: 64 consecutive graphs per core (512 graphs / 8 cores).  Graphs are
grouped into octs of 8; each oct's nodes are padded to a fixed number of
128-node tiles (T) so every core runs the identical instruction stream.
Weights are replicated; per-graph outputs are disjoint, so the host simply
concatenates the 8 core outputs.

On-core algorithm (single pass over x, bf16 compute, fp32 accumulate):
  - x^T tiles arrive via DMA xbar-transpose (no TensorE transpose cost)
  - z^T = W1^T @ x^T on TensorE; tanh(+bias) on ScalarE -> t^T
  - s (node-major, replicated 8x across oct slots) = t^T-stationary matmuls
  - e = exp(s) on ScalarE; sel = e * mask (host-built 0/1 oct masks) on DVE
  - pooled[(oct,graph,head), 0:256] += sel^T @ [x | 1]  accumulated in PSUM;
    column 256 (the ones column) gives the softmax denominator
  - epilogue: divide by denominator, average heads with a constant matmul
"""

import numpy as np
import ml_dtypes

BF16 = ml_dtypes.bfloat16

N_CORES = 8
H = 256
HEADS = 4
GRP = 8  # graphs per oct group
SELW = GRP * HEADS  # 32 selector columns per node

_NC_CACHE = {}
LAST_RESULT = None


def _build_nc(T: int, n_grps: int):
    """Build the SPMD Bass program. T = 128-node tiles per oct group."""
    import concourse.bacc as bacc
    import concourse.mybir as mybir
    from concourse.tile import TileContext

    fp32 = mybir.dt.float32
    bf16 = mybir.dt.bfloat16
    AF = mybir.ActivationFunctionType

    n_pad = n_grps * T * 128
    n_tiles = n_grps * T
    assert n_tiles % 4 == 0, "groups*T must be a multiple of 4 (512-node supertiles)"
    n_supers = n_tiles // 4
    assert n_grps == 8, "psum layout assumes 8 octs (64 graphs) per core"

    nc = bacc.Bacc(trn_type="TRN2")

    xa = nc.dram_tensor("xa", [n_pad, H + 1], bf16, kind="ExternalInput")
    xt = nc.dram_tensor("xt", [H, n_pad], bf16, kind="ExternalInput")
    msk = nc.dram_tensor("msk", [n_pad, SELW], bf16, kind="ExternalInput")
    w1 = nc.dram_tensor("w1", [128, 512], bf16, kind="ExternalInput")
    w2 = nc.dram_tensor("w2", [128, 2 * SELW], bf16, kind="ExternalInput")
    b1d = nc.dram_tensor("b1d", [128, 2], fp32, kind="ExternalInput")
    shs = nc.dram_tensor("shs", [128, 32], bf16, kind="ExternalInput")
    out = nc.dram_tensor("out", [64, H], fp32, kind="ExternalOutput")

    xa_r = xa.ap().rearrange("(s j p) c -> s p j c", j=4, p=128)
    msk_r = msk.ap().rearrange("(s j p) c -> s p j c", j=4, p=128)

    with TileContext(nc) as tc:
        with (
            tc.tile_pool(name="consts", bufs=1) as cpool,
            tc.tile_pool(name="acc", bufs=1, space="PSUM") as acc_pool,
        ):
            w1_sb = cpool.tile([128, 512], bf16)
            w2_sb = cpool.tile([128, 2 * SELW], bf16)
            b1_sb = cpool.tile([128, 2], fp32)
            shs_sb = cpool.tile([128, 32], bf16)
            nc.sync.dma_start(w1_sb[:], w1.ap())
            nc.sync.dma_start(w2_sb[:], w2.ap())
            nc.sync.dma_start(b1_sb[:], b1d.ap())
            nc.sync.dma_start(shs_sb[:], shs.ap())

            # persistent accumulators: rows = (oct%4)*32 + jj*4 + h, col 256 = seg_e
            poolA = acc_pool.tile([128, H + 1], fp32)
            poolB = acc_pool.tile([128, H + 1], fp32)

            with (
                tc.tile_pool(name="data", bufs=3) as dpool,
                tc.tile_pool(name="work", bufs=2) as wpool,
                tc.tile_pool(name="mm", bufs=2, space="PSUM") as mpool,
            ):
                for sup in range(n_supers):
                    rows = slice(sup * 512, (sup + 1) * 512)

                    xa_sb = dpool.tile([128, 4 * (H + 1)], bf16, name="xa_sb")
                    xt0 = dpool.tile([128, 512], bf16, name="xt0")
                    xt1 = dpool.tile([128, 512], bf16, name="xt1")
                    msk_sb = dpool.tile([128, 4 * SELW], bf16, name="msk_sb")

                    nc.sync.dma_start(
                        xa_sb[:].rearrange("p (j c) -> p j c", j=4), xa_r[sup]
                    )
                    nc.sync.dma_start(xt0[:], xt.ap()[0:128, rows])
                    nc.sync.dma_start(xt1[:], xt.ap()[128:256, rows])
                    nc.sync.dma_start(
                        msk_sb[:].rearrange("p (j c) -> p j c", j=4), msk_r[sup]
                    )

                    # z^T = W1^T @ x^T  (two H_out chunks), tanh(+bias) -> t^T
                    tts = []
                    for ko in range(2):
                        zt = mpool.tile([128, 512], fp32, name=f"zt{ko}", tag=f"zt{ko}")
                        for ki in range(2):
                            nc.tensor.matmul(
                                zt[:],
                                w1_sb[:, ki * 256 + ko * 128 : ki * 256 + ko * 128 + 128],
                                (xt0 if ki == 0 else xt1)[:],
                                start=(ki == 0),
                                stop=(ki == 1),
                            )
                        tt = wpool.tile([128, 512], bf16, name=f"tt{ko}", tag=f"tt{ko}")
                        nc.scalar.activation(
                            tt[:], zt[:], AF.Tanh, bias=b1_sb[:, ko : ko + 1]
                        )
                        tts.append(tt)

                    # s (node-major, replicated over the 8 oct slots): [128, 4*SELW]
                    s_ps = mpool.tile([128, 4 * SELW], fp32, name="s_ps", tag="s_ps")
                    for j in range(4):
                        for ko in range(2):
                            nc.tensor.matmul(
                                s_ps[:, j * SELW : (j + 1) * SELW],
                                tts[ko][:, j * 128 : (j + 1) * 128],
                                w2_sb[:, ko * SELW : (ko + 1) * SELW],
                                start=(ko == 0),
                                stop=(ko == 1),
                            )
                    e_sb = wpool.tile([128, 4 * SELW], bf16, name="e_sb")
                    nc.scalar.activation(e_sb[:], s_ps[:], AF.Exp)

                    # selector = e * oct-mask  (both [128, 4*SELW])
                    sel = wpool.tile([128, 4 * SELW], bf16, name="sel")
                    nc.vector.tensor_tensor(
                        sel[:], e_sb[:], msk_sb[:], mybir.AluOpType.mult
                    )

                    # pooled[(o%4)*32 : +32, :] += sel_j^T @ [x_j | 1]
                    for j in range(4):
                        t_glob = sup * 4 + j
                        o = t_glob // T
                        tau = t_glob % T
                        ps = poolA if (o % 8) < 4 else poolB
                        r0 = (o % 4) * 32
                        nc.tensor.matmul(
                            ps[r0 : r0 + 32, :],
                            sel[:, j * SELW : (j + 1) * SELW],
                            xa_sb[:, j * (H + 1) : (j + 1) * (H + 1)],
                            start=(tau == 0),
                            stop=(tau == T - 1),
                            tile_position=(0, r0),
                        )

            # epilogue: normalize by seg_e, mean over heads via constant matmul
            with (
                tc.tile_pool(name="ep", bufs=1) as ep,
                tc.tile_pool(name="epp", bufs=1, space="PSUM") as epp,
            ):
                for idx, ps in enumerate([poolA, poolB]):
                    rec = ep.tile([128, 1], fp32, name=f"rec{idx}")
                    nc.vector.reciprocal(rec[:], ps[:, H : H + 1])
                    norm = ep.tile([128, H], bf16, name=f"norm{idx}")
                    nc.vector.tensor_scalar(
                        norm[:], ps[:, 0:H], rec[:], None, mybir.AluOpType.mult
                    )
                    fin = epp.tile([32, H], fp32, name=f"fin{idx}")
                    nc.tensor.matmul(fin[:], shs_sb[:], norm[:], start=True, stop=True)
                    osb = ep.tile([32, H], fp32, name=f"osb{idx}")
                    nc.scalar.copy(osb[:], fin[:])
                    nc.sync.dma_start(out.ap()[idx * 32 : (idx + 1) * 32, :], osb[:])

    nc.finalize()
    return nc


def _host_prep(x, batch, W1, b1, W2, G):
    """Shard + pad inputs; build all per-core DRAM arrays."""
    gpc = G // N_CORES  # graphs per core
    n_grps = gpc // GRP  # oct groups per core
    counts = np.bincount(batch, minlength=G)
    grp_sums = counts.reshape(-1, GRP).sum(axis=1)  # [G/8]
    T = int(np.ceil(grp_sums.max() / 128))
    grp_nodes = T * 128
    n_pad = n_grps * grp_nodes

    starts = np.zeros(G + 1, dtype=np.int64)
    np.cumsum(counts, out=starts[1:])

    x_bf = x.astype(BF16)
    xa = np.zeros((N_CORES, n_pad, H + 1), dtype=BF16)
    msk = np.zeros((N_CORES, n_pad, SELW), dtype=BF16)
    jj_of_col = (np.arange(SELW) // HEADS).astype(np.int64)  # [SELW] -> oct slot

    for c in range(N_CORES):
        for gl in range(n_grps):
            o = c * n_grps + gl
            g0 = o * GRP
            s, e = int(starts[g0]), int(starts[g0 + GRP])
            cnt = e - s
            base = gl * grp_nodes
            xa[c, base : base + cnt, 0:H] = x_bf[s:e]
            xa[c, base : base + cnt, H] = BF16(1.0)
            bloc = (batch[s:e] - g0).astype(np.int64)  # 0..7 within oct
            msk[c, base : base + cnt, :] = (
                bloc[:, None] == jj_of_col[None, :]
            ).astype(BF16)

    xth = np.ascontiguousarray(xa[:, :, 0:H].transpose(0, 2, 1))  # [cores, H, n_pad]

    # w1 blocks: [:, ki*256 + 0:256] = W1[ki*128:(ki+1)*128, :]
    w1h = np.zeros((128, 512), dtype=BF16)
    w1h[:, 0:256] = W1[0:128, :].astype(BF16)
    w1h[:, 256:512] = W1[128:256, :].astype(BF16)
    # w2 replicated over the 8 oct slots: [:, ko*SELW + jj*4 + h] = W2[ko*128+p, h]
    w2h = np.zeros((128, 2 * SELW), dtype=BF16)
    for ko in range(2):
        blk = W2[ko * 128 : (ko + 1) * 128, :].astype(BF16)  # [128, 4]
        w2h[:, ko * SELW : (ko + 1) * SELW] = np.tile(blk, (1, GRP))
    b1h = np.stack([b1[0:128], b1[128:256]], axis=1).astype(np.float32)  # [128, 2]
    # head-mean matrix: rows p=(o%4)*32+jj*4+h -> graph column p//4, value 1/4
    shsh = np.zeros((128, 32), dtype=BF16)
    shsh[np.arange(128), np.arange(128) // HEADS] = BF16(0.25)

    return T, n_grps, xa, xth, msk, w1h, w2h, b1h, shsh


def kernel(x, batch, W1, b1, W2, num_graphs):
    global LAST_RESULT
    from concourse.bass_utils import run_bass_kernel_spmd

    x = np.asarray(x, dtype=np.float32)
    batch = np.asarray(batch).astype(np.int64)
    W1 = np.asarray(W1, dtype=np.float32)
    b1 = np.asarray(b1, dtype=np.float32)
    W2 = np.asarray(W2, dtype=np.float32)
    G = int(num_graphs)

    T, n_grps, xa, xth, msk, w1h, w2h, b1h, shsh = _host_prep(
        x, batch, W1, b1, W2, G
    )

    key = (T, n_grps)
    if key not in _NC_CACHE:
        _NC_CACHE[key] = _build_nc(T, n_grps)
    nc = _NC_CACHE[key]

    in_maps = [
        {
            "xa": xa[c],
            "xt": xth[c],
            "msk": msk[c],
            "w1": w1h,
            "w2": w2h,
            "b1d": b1h,
            "shs": shsh,
        }
        for c in range(N_CORES)
    ]

    import os

    trace = bool(int(os.environ.get("KERNEL_TRACE", "0")))
    res = run_bass_kernel_spmd(
        nc, in_maps, core_ids=list(range(N_CORES)), trace=trace
    )
    LAST_RESULT = res
    return np.concatenate([res.results[c]["out"] for c in range(N_CORES)], axis=0)


# revision 18
# speedup vs baseline: 36.8089x; 36.8089x over previous
"""AttentionPooling Trainium2 kernel (8 NeuronCores, SPMD).

Reference computation:
    scores = tanh(x @ W1 + b1) @ W2          # [N, 4]
    w      = segment_softmax(scores, batch)  # per-graph softmax over nodes
    out[g] = mean_h( sum_{n in g} w[n,h] * x[n] )   # [G, 256]

Sharding: 64 consecutive graphs per core (512 graphs / 8 cores).  Graphs are
grouped into octs of 8; each oct's nodes are padded to a fixed number of
128-node tiles (T) so every core runs the identical instruction stream.
Weights are replicated; per-graph outputs are disjoint, so the host simply
concatenates the 8 core outputs.

On-core algorithm (single pass over x, bf16 compute, fp32 accumulate):
  - packed row [x | 1 | bloc | pad] and a host-pretransposed x^T stream in
    1024-node DMA chunks (2 DMA instructions per chunk)
  - z^T = W1^T @ x^T on TensorE; tanh(+bias) on ScalarE -> t^T
  - s (node-major, replicated 8x across oct slots) = t^T-stationary matmuls
  - e = exp(s) on ScalarE; oct one-hot mask built on DVE from bloc via
    iota-compare; sel = e * mask on DVE
  - pooled[(oct,graph,head), 0:256] += sel^T @ [x | 1]  accumulated in PSUM;
    column 256 (the ones column) gives the softmax denominator
  - epilogue: divide by denominator, average heads with a constant matmul
"""

import numpy as np
import ml_dtypes

BF16 = ml_dtypes.bfloat16

N_CORES = 8
H = 256
HEADS = 4
GRP = 8  # graphs per oct group
SELW = GRP * HEADS  # 32 selector columns per node
ROW = H + 4  # packed row: x(256) | ones(1) | bloc(1) | pad(2)
BLOC = H + 1  # bloc column index

_NC_CACHE = {}
LAST_RESULT = None


def _build_nc(T: int, n_grps: int, repeats: int = 1):
    """Build the SPMD Bass program. T = 128-node tiles per oct group."""
    import concourse.bacc as bacc
    import concourse.mybir as mybir
    from concourse.tile import TileContext

    fp32 = mybir.dt.float32
    bf16 = mybir.dt.bfloat16
    AF = mybir.ActivationFunctionType

    n_pad = n_grps * T * 128
    n_tiles = n_grps * T
    n_supers = n_tiles // 4
    assert n_supers % 2 == 0
    n_chunks = n_supers // 2  # 1024-node DMA chunks
    assert n_grps == 8, "psum layout assumes 8 octs (64 graphs) per core"

    nc = bacc.Bacc(trn_type="TRN2")

    xam = nc.dram_tensor("xam", [n_chunks, 128, 8 * ROW], bf16, kind="ExternalInput")
    xt = nc.dram_tensor("xt", [n_chunks, 128, 2 * 1024], bf16, kind="ExternalInput")
    w1 = nc.dram_tensor("w1", [128, 512], bf16, kind="ExternalInput")
    w2 = nc.dram_tensor("w2", [128, 2 * HEADS], bf16, kind="ExternalInput")
    b1d = nc.dram_tensor("b1d", [128, 2], fp32, kind="ExternalInput")
    shs = nc.dram_tensor("shs", [128, 32], bf16, kind="ExternalInput")
    iot = nc.dram_tensor("iot", [128, SELW], bf16, kind="ExternalInput")
    out = nc.dram_tensor("out", [64, H], fp32, kind="ExternalOutput")



    with TileContext(nc) as tc:
        with (
            tc.tile_pool(name="consts", bufs=1) as cpool,
            tc.tile_pool(name="acc", bufs=1, space="PSUM") as acc_pool,
        ):
            w1_sb = cpool.tile([128, 512], bf16)
            w2_sb = cpool.tile([128, 2 * HEADS], bf16)
            b1_sb = cpool.tile([128, 2], fp32)
            shs_sb = cpool.tile([128, 32], bf16)
            iot_sb = cpool.tile([128, SELW], bf16)
            nc.sync.dma_start(w1_sb[:], w1.ap())
            nc.sync.dma_start(w2_sb[:], w2.ap())
            nc.sync.dma_start(b1_sb[:], b1d.ap())
            nc.sync.dma_start(shs_sb[:], shs.ap())
            nc.sync.dma_start(iot_sb[:], iot.ap())

            # persistent accumulators: rows = (oct%4)*32 + jj*4 + h, col 256 = seg_e
            poolA = acc_pool.tile([128, H + 1], fp32)
            poolB = acc_pool.tile([128, H + 1], fp32)

            with (
                tc.tile_pool(name="data", bufs=6) as dpool,
                tc.tile_pool(name="work", bufs=3) as wpool,
                tc.tile_pool(name="mm", bufs=2, space="PSUM") as mpool,
            ):
              for _rep in range(repeats):
                for ch in range(n_chunks):
                    xam_sb = dpool.tile([128, 8 * ROW], bf16, name="xam_sb")
                    xt_sb = dpool.tile([128, 2 * 1024], bf16, name="xt_sb")
                    nc.sync.dma_start(xam_sb[:], xam.ap()[ch])
                    nc.sync.dma_start(xt_sb[:], xt.ap()[ch])

                    for s2 in range(2):
                        # z^T = W1^T @ x^T (two H_out chunks into one psum tile)
                        zt = mpool.tile([128, 1024], fp32, name="zt", tag="zt")
                        for ko in range(2):
                            for ki in range(2):
                                nc.tensor.matmul(
                                    zt[:, ko * 512 : (ko + 1) * 512],
                                    w1_sb[
                                        :,
                                        ki * 256 + ko * 128 : ki * 256 + ko * 128 + 128,
                                    ],
                                    xt_sb[
                                        :,
                                        ki * 1024 + s2 * 512 : ki * 1024 + s2 * 512 + 512,
                                    ],
                                    start=(ki == 0),
                                    stop=(ki == 1),
                                )
                        # tanh(+bias): bias is per-partition, one op per H_out chunk
                        tt = wpool.tile([128, 1024], bf16, name="tt", tag="tt")
                        for ko in range(2):
                            nc.scalar.activation(
                                tt[:, ko * 512 : (ko + 1) * 512],
                                zt[:, ko * 512 : (ko + 1) * 512],
                                AF.Tanh,
                                bias=b1_sb[:, ko : ko + 1],
                            )

                        # s (node-major): [128, 4 tiles * 4 heads]
                        s_ps = mpool.tile([128, 4 * HEADS], fp32, name="s_ps", tag="s_ps")
                        for j in range(4):
                            for ko in range(2):
                                nc.tensor.matmul(
                                    s_ps[:, j * HEADS : (j + 1) * HEADS],
                                    tt[:, ko * 512 + j * 128 : ko * 512 + j * 128 + 128],
                                    w2_sb[:, ko * HEADS : (ko + 1) * HEADS],
                                    start=(ko == 0),
                                    stop=(ko == 1),
                                )
                        e_sb = wpool.tile([128, 4 * HEADS], bf16, name="e_sb")
                        nc.scalar.activation(e_sb[:], s_ps[:], AF.Exp)

                        # oct one-hot masks: (bloc == iota), one broadcast op
                        jbase = s2 * 4
                        mk = wpool.tile([128, 4 * SELW], bf16, name="mk")
                        bloc_b = (
                            xam_sb[:]
                            .rearrange("p (j c) -> p j c", j=8)[
                                :, jbase : jbase + 4, BLOC : BLOC + 1
                            ]
                            .broadcast_to((128, 4, SELW))
                        )
                        iot_b = (
                            iot_sb[:]
                            .rearrange("p (o c) -> p o c", o=1)
                            .broadcast_to((128, 4, SELW))
                        )
                        nc.vector.tensor_tensor(
                            mk[:].rearrange("p (j c) -> p j c", j=4),
                            bloc_b,
                            iot_b,
                            mybir.AluOpType.is_equal,
                        )
                        # selector = e * mask (e broadcast over the 8 oct slots)
                        sel = wpool.tile([128, 4 * SELW], bf16, name="sel")
                        e_b = (
                            e_sb[:]
                            .rearrange("p (j o h) -> p j o h", j=4, o=1)
                            .broadcast_to((128, 4, GRP, HEADS))
                        )
                        nc.vector.tensor_tensor(
                            sel[:].rearrange("p (j o h) -> p j o h", j=4, o=GRP),
                            e_b,
                            mk[:].rearrange("p (j o h) -> p j o h", j=4, o=GRP),
                            mybir.AluOpType.mult,
                        )

                        # pooled[(o%4)*32 : +32, :] += sel_j^T @ [x_j | 1]
                        for j in range(4):
                            t_glob = (ch * 2 + s2) * 4 + j
                            o = t_glob // T
                            tau = t_glob % T
                            ps = poolA if (o % 8) < 4 else poolB
                            r0 = (o % 4) * 32
                            jj = jbase + j
                            nc.tensor.matmul(
                                ps[r0 : r0 + 32, :],
                                sel[:, j * SELW : (j + 1) * SELW],
                                xam_sb[:, jj * ROW : jj * ROW + H + 1],
                                start=(tau == 0),
                                stop=(tau == T - 1),
                                tile_position=(0, r0),
                            )

            # epilogue: normalize by seg_e, mean over heads via constant matmul
            with (
                tc.tile_pool(name="ep", bufs=1) as ep,
                tc.tile_pool(name="epp", bufs=1, space="PSUM") as epp,
            ):
                for idx, ps in enumerate([poolA, poolB]):
                    # clamp seg_e away from 0 so empty graphs yield 0, not NaN
                    seg = ep.tile([128, 1], fp32, name=f"seg{idx}")
                    nc.vector.tensor_scalar(
                        seg[:], ps[:, H : H + 1], 1e-30, None, mybir.AluOpType.max
                    )
                    rec = ep.tile([128, 1], fp32, name=f"rec{idx}")
                    nc.vector.reciprocal(rec[:], seg[:])
                    norm = ep.tile([128, H], bf16, name=f"norm{idx}")
                    nc.vector.tensor_scalar(
                        norm[:], ps[:, 0:H], rec[:], None, mybir.AluOpType.mult
                    )
                    fin = epp.tile([32, H], fp32, name=f"fin{idx}")
                    nc.tensor.matmul(fin[:], shs_sb[:], norm[:], start=True, stop=True)
                    osb = ep.tile([32, H], fp32, name=f"osb{idx}")
                    nc.scalar.copy(osb[:], fin[:])
                    nc.sync.dma_start(out.ap()[idx * 32 : (idx + 1) * 32, :], osb[:])

    nc.finalize()
    return nc


def _host_prep(x, batch, W1, b1, W2, G):
    """Shard + pad inputs; build all per-core DRAM arrays."""
    gpc = G // N_CORES  # graphs per core
    n_grps = gpc // GRP  # oct groups per core
    counts = np.bincount(batch, minlength=G)
    grp_sums = counts.reshape(-1, GRP).sum(axis=1)  # [G/8]
    T = int(np.ceil(grp_sums.max() / 128))
    grp_nodes = T * 128
    n_pad = n_grps * grp_nodes

    starts = np.zeros(G + 1, dtype=np.int64)
    np.cumsum(counts, out=starts[1:])

    x_bf = x.astype(BF16)
    xam = np.zeros((N_CORES, n_pad, ROW), dtype=BF16)
    for c in range(N_CORES):
        xam[c, :, BLOC] = BF16(-1.0)  # padding nodes match no oct slot
    for c in range(N_CORES):
        for gl in range(n_grps):
            o = c * n_grps + gl
            g0 = o * GRP
            s, e = int(starts[g0]), int(starts[g0 + GRP])
            cnt = e - s
            base = gl * grp_nodes
            xam[c, base : base + cnt, 0:H] = x_bf[s:e]
            xam[c, base : base + cnt, H] = BF16(1.0)
            xam[c, base : base + cnt, BLOC] = (batch[s:e] - g0).astype(BF16)

    n_chunks = n_pad // 1024
    # chunk-major contiguous layouts: one 4KB+ read per partition per chunk
    xth = xam[:, :, 0:H].transpose(0, 2, 1)  # [cores, H, n_pad]
    xt2 = np.ascontiguousarray(
        xth.reshape(N_CORES, 2, 128, n_chunks, 1024)
        .transpose(0, 3, 2, 1, 4)
        .reshape(N_CORES, n_chunks, 128, 2 * 1024)
    )
    xam2 = np.ascontiguousarray(
        xam.reshape(N_CORES, n_chunks, 8, 128, ROW)
        .transpose(0, 1, 3, 2, 4)
        .reshape(N_CORES, n_chunks, 128, 8 * ROW)
    )

    # w1 blocks: [:, ki*256 + 0:256] = W1[ki*128:(ki+1)*128, :]
    w1h = np.zeros((128, 512), dtype=BF16)
    w1h[:, 0:256] = W1[0:128, :].astype(BF16)
    w1h[:, 256:512] = W1[128:256, :].astype(BF16)
    # w2 chunks: [:, ko*HEADS : +HEADS] = W2[ko*128:(ko+1)*128, :]
    w2h = np.zeros((128, 2 * HEADS), dtype=BF16)
    for ko in range(2):
        w2h[:, ko * HEADS : (ko + 1) * HEADS] = W2[
            ko * 128 : (ko + 1) * 128, :
        ].astype(BF16)
    b1h = np.stack([b1[0:128], b1[128:256]], axis=1).astype(np.float32)  # [128, 2]
    # head-mean matrix: rows p=(o%4)*32+jj*4+h -> graph column p//4, value 1/4
    shsh = np.zeros((128, 32), dtype=BF16)
    shsh[np.arange(128), np.arange(128) // HEADS] = BF16(0.25)
    # iota over oct slots, one value per selector column, bcast to all partitions
    ioth = np.broadcast_to(
        (np.arange(SELW) // HEADS).astype(BF16)[None, :], (128, SELW)
    ).copy()

    return T, n_grps, xam2, xt2, w1h, w2h, b1h, shsh, ioth


def kernel(x, batch, W1, b1, W2, num_graphs):
    global LAST_RESULT
    from concourse.bass_utils import run_bass_kernel_spmd

    x = np.asarray(x, dtype=np.float32)
    batch = np.asarray(batch).astype(np.int64)
    W1 = np.asarray(W1, dtype=np.float32)
    b1 = np.asarray(b1, dtype=np.float32)
    W2 = np.asarray(W2, dtype=np.float32)
    G = int(num_graphs)

    T, n_grps, xam, xth, w1h, w2h, b1h, shsh, ioth = _host_prep(
        x, batch, W1, b1, W2, G
    )

    key = (T, n_grps)
    if key not in _NC_CACHE:
        _NC_CACHE[key] = _build_nc(T, n_grps)
    nc = _NC_CACHE[key]

    in_maps = [
        {
            "xam": xam[c],
            "xt": xth[c],
            "w1": w1h,
            "w2": w2h,
            "b1d": b1h,
            "shs": shsh,
            "iot": ioth,
        }
        for c in range(N_CORES)
    ]

    res = run_bass_kernel_spmd(nc, in_maps, core_ids=list(range(N_CORES)))
    LAST_RESULT = res
    return np.concatenate([res.results[c]["out"] for c in range(N_CORES)], axis=0)


# revision 19
# speedup vs baseline: 141.7814x; 3.8518x over previous
"""AttentionPooling Trainium2 kernel (8 NeuronCores, SPMD).

Reference computation:
    scores = tanh(x @ W1 + b1) @ W2          # [N, 4]
    w      = segment_softmax(scores, batch)  # per-graph softmax over nodes
    out[g] = mean_h( sum_{n in g} w[n,h] * x[n] )   # [G, 256]

Sharding: 64 consecutive graphs per core (512 graphs / 8 cores).  Graphs are
grouped into octs of 8; each oct's nodes are padded to a fixed number of
128-node tiles (T) so every core runs the identical instruction stream.
Weights are replicated; per-graph outputs are disjoint, so the host simply
concatenates the 8 core outputs.

On-core algorithm (single pass over x, bf16 compute, fp32 accumulate):
  - packed row [x | 1 | bloc | pad] and a host-pretransposed x^T stream in
    1024-node DMA chunks (2 DMA instructions per chunk)
  - z^T = W1^T @ x^T on TensorE; tanh(+bias) on ScalarE -> t^T
  - s (node-major, replicated 8x across oct slots) = t^T-stationary matmuls
  - e = exp(s) on ScalarE; oct one-hot mask built on DVE from bloc via
    iota-compare; sel = e * mask on DVE
  - pooled[(oct,graph,head), 0:256] += sel^T @ [x | 1]  accumulated in PSUM;
    column 256 (the ones column) gives the softmax denominator
  - epilogue: divide by denominator, average heads with a constant matmul
"""

import numpy as np
import ml_dtypes

BF16 = ml_dtypes.bfloat16

N_CORES = 8
H = 256
HEADS = 4
GRP = 8  # graphs per oct group
SELW = GRP * HEADS  # 32 selector columns per node
ROW = H + 4  # packed row: x(256) | ones(1) | bloc(1) | pad(2)
BLOC = H + 1  # bloc column index

_NC_CACHE = {}
LAST_RESULT = None


def _build_nc(T: int, n_grps: int, repeats: int = 1):
    """Build the SPMD Bass program. T = 128-node tiles per oct group."""
    import concourse.bacc as bacc
    import concourse.mybir as mybir
    from concourse.tile import TileContext

    fp32 = mybir.dt.float32
    bf16 = mybir.dt.bfloat16
    AF = mybir.ActivationFunctionType

    n_pad = n_grps * T * 128
    n_tiles = n_grps * T
    n_supers = n_tiles // 4
    assert n_supers % 2 == 0
    n_chunks = n_supers // 2  # 1024-node DMA chunks
    assert n_grps == 8, "psum layout assumes 8 octs (64 graphs) per core"

    nc = bacc.Bacc(trn_type="TRN2")

    xam = nc.dram_tensor("xam", [n_chunks, 128, 8 * ROW], bf16, kind="ExternalInput")
    xt = nc.dram_tensor("xt", [n_chunks, 128, 2 * 1024], bf16, kind="ExternalInput")
    w1 = nc.dram_tensor("w1", [128, 512], bf16, kind="ExternalInput")
    w2 = nc.dram_tensor("w2", [128, 2 * HEADS], bf16, kind="ExternalInput")
    b1d = nc.dram_tensor("b1d", [128, 2], fp32, kind="ExternalInput")
    shs = nc.dram_tensor("shs", [128, 32], bf16, kind="ExternalInput")
    iot = nc.dram_tensor("iot", [128, SELW], bf16, kind="ExternalInput")
    out = nc.dram_tensor("out", [64, H], fp32, kind="ExternalOutput")



    with TileContext(nc) as tc:
        with (
            tc.tile_pool(name="consts", bufs=1) as cpool,
            tc.tile_pool(name="acc", bufs=1, space="PSUM") as acc_pool,
        ):
            w1_sb = cpool.tile([128, 512], bf16)
            w2_sb = cpool.tile([128, 2 * HEADS], bf16)
            b1_sb = cpool.tile([128, 2], fp32)
            shs_sb = cpool.tile([128, 32], bf16)
            iot_sb = cpool.tile([128, SELW], bf16)
            nc.sync.dma_start(w1_sb[:], w1.ap())
            nc.sync.dma_start(w2_sb[:], w2.ap())
            nc.sync.dma_start(b1_sb[:], b1d.ap())
            nc.sync.dma_start(shs_sb[:], shs.ap())
            nc.sync.dma_start(iot_sb[:], iot.ap())

            # persistent accumulators: rows = (oct%4)*32 + jj*4 + h, col 256 = seg_e
            poolA = acc_pool.tile([128, H + 1], fp32)
            poolB = acc_pool.tile([128, H + 1], fp32)

            with (
                tc.tile_pool(name="data", bufs=8) as dpool,
                tc.tile_pool(name="work", bufs=4) as wpool,
                tc.tile_pool(name="mm", bufs=2, space="PSUM") as mpool,
            ):
              for _rep in range(repeats):
                for ch in range(n_chunks):
                    xam_sb = dpool.tile([128, 8 * ROW], bf16, name="xam_sb")
                    xt_sb = dpool.tile([128, 2 * 1024], bf16, name="xt_sb")
                    nc.sync.dma_start(xam_sb[:], xam.ap()[ch])
                    nc.gpsimd.dma_start(xt_sb[:], xt.ap()[ch])

                    for s2 in range(2):
                        # z^T = W1^T @ x^T (two H_out chunks into one psum tile)
                        zt = mpool.tile([128, 1024], fp32, name="zt", tag="zt")
                        for ko in range(2):
                            for ki in range(2):
                                nc.tensor.matmul(
                                    zt[:, ko * 512 : (ko + 1) * 512],
                                    w1_sb[
                                        :,
                                        ki * 256 + ko * 128 : ki * 256 + ko * 128 + 128,
                                    ],
                                    xt_sb[
                                        :,
                                        ki * 1024 + s2 * 512 : ki * 1024 + s2 * 512 + 512,
                                    ],
                                    start=(ki == 0),
                                    stop=(ki == 1),
                                )
                        # tanh(+bias): bias is per-partition, one op per H_out chunk
                        tt = wpool.tile([128, 1024], bf16, name="tt", tag="tt")
                        for ko in range(2):
                            nc.scalar.activation(
                                tt[:, ko * 512 : (ko + 1) * 512],
                                zt[:, ko * 512 : (ko + 1) * 512],
                                AF.Tanh,
                                bias=b1_sb[:, ko : ko + 1],
                            )

                        # s (node-major): [128, 4 tiles * 4 heads]
                        s_ps = mpool.tile([128, 4 * HEADS], fp32, name="s_ps", tag="s_ps")
                        for j in range(4):
                            for ko in range(2):
                                nc.tensor.matmul(
                                    s_ps[:, j * HEADS : (j + 1) * HEADS],
                                    tt[:, ko * 512 + j * 128 : ko * 512 + j * 128 + 128],
                                    w2_sb[:, ko * HEADS : (ko + 1) * HEADS],
                                    start=(ko == 0),
                                    stop=(ko == 1),
                                )
                        e_sb = wpool.tile([128, 4 * HEADS], bf16, name="e_sb")
                        nc.scalar.activation(e_sb[:], s_ps[:], AF.Exp)

                        # oct one-hot masks: (bloc == iota), one broadcast op
                        jbase = s2 * 4
                        mk = wpool.tile([128, 4 * SELW], bf16, name="mk")
                        bloc_b = (
                            xam_sb[:]
                            .rearrange("p (j c) -> p j c", j=8)[
                                :, jbase : jbase + 4, BLOC : BLOC + 1
                            ]
                            .broadcast_to((128, 4, SELW))
                        )
                        iot_b = (
                            iot_sb[:]
                            .rearrange("p (o c) -> p o c", o=1)
                            .broadcast_to((128, 4, SELW))
                        )
                        nc.vector.tensor_tensor(
                            mk[:].rearrange("p (j c) -> p j c", j=4),
                            bloc_b,
                            iot_b,
                            mybir.AluOpType.is_equal,
                        )
                        # selector = e * mask (e broadcast over the 8 oct slots)
                        sel = wpool.tile([128, 4 * SELW], bf16, name="sel")
                        e_b = (
                            e_sb[:]
                            .rearrange("p (j o h) -> p j o h", j=4, o=1)
                            .broadcast_to((128, 4, GRP, HEADS))
                        )
                        nc.vector.tensor_tensor(
                            sel[:].rearrange("p (j o h) -> p j o h", j=4, o=GRP),
                            e_b,
                            mk[:].rearrange("p (j o h) -> p j o h", j=4, o=GRP),
                            mybir.AluOpType.mult,
                        )

                        # pooled[(o%4)*32 : +32, :] += sel_j^T @ [x_j | 1]
                        for j in range(4):
                            t_glob = (ch * 2 + s2) * 4 + j
                            o = t_glob // T
                            tau = t_glob % T
                            ps = poolA if (o % 8) < 4 else poolB
                            r0 = (o % 4) * 32
                            jj = jbase + j
                            nc.tensor.matmul(
                                ps[r0 : r0 + 32, :],
                                sel[:, j * SELW : (j + 1) * SELW],
                                xam_sb[:, jj * ROW : jj * ROW + H + 1],
                                start=(tau == 0),
                                stop=(tau == T - 1),
                                tile_position=(0, r0),
                            )

            # epilogue: normalize by seg_e, mean over heads via constant matmul
            with (
                tc.tile_pool(name="ep", bufs=1) as ep,
                tc.tile_pool(name="epp", bufs=1, space="PSUM") as epp,
            ):
                for idx, ps in enumerate([poolA, poolB]):
                    # clamp seg_e away from 0 so empty graphs yield 0, not NaN
                    seg = ep.tile([128, 1], fp32, name=f"seg{idx}")
                    nc.vector.tensor_scalar(
                        seg[:], ps[:, H : H + 1], 1e-30, None, mybir.AluOpType.max
                    )
                    rec = ep.tile([128, 1], fp32, name=f"rec{idx}")
                    nc.vector.reciprocal(rec[:], seg[:])
                    norm = ep.tile([128, H], bf16, name=f"norm{idx}")
                    nc.vector.tensor_scalar(
                        norm[:], ps[:, 0:H], rec[:], None, mybir.AluOpType.mult
                    )
                    fin = epp.tile([32, H], fp32, name=f"fin{idx}")
                    nc.tensor.matmul(fin[:], shs_sb[:], norm[:], start=True, stop=True)
                    osb = ep.tile([32, H], fp32, name=f"osb{idx}")
                    nc.scalar.copy(osb[:], fin[:])
                    nc.sync.dma_start(out.ap()[idx * 32 : (idx + 1) * 32, :], osb[:])

    nc.finalize()
    return nc


def _host_prep(x, batch, W1, b1, W2, G):
    """Shard + pad inputs; build all per-core DRAM arrays."""
    gpc = G // N_CORES  # graphs per core
    n_grps = gpc // GRP  # oct groups per core
    counts = np.bincount(batch, minlength=G)
    grp_sums = counts.reshape(-1, GRP).sum(axis=1)  # [G/8]
    T = int(np.ceil(grp_sums.max() / 128))
    grp_nodes = T * 128
    n_pad = n_grps * grp_nodes

    starts = np.zeros(G + 1, dtype=np.int64)
    np.cumsum(counts, out=starts[1:])

    x_bf = x.astype(BF16)
    xam = np.zeros((N_CORES, n_pad, ROW), dtype=BF16)
    for c in range(N_CORES):
        xam[c, :, BLOC] = BF16(-1.0)  # padding nodes match no oct slot
    for c in range(N_CORES):
        for gl in range(n_grps):
            o = c * n_grps + gl
            g0 = o * GRP
            s, e = int(starts[g0]), int(starts[g0 + GRP])
            cnt = e - s
            base = gl * grp_nodes
            xam[c, base : base + cnt, 0:H] = x_bf[s:e]
            xam[c, base : base + cnt, H] = BF16(1.0)
            xam[c, base : base + cnt, BLOC] = (batch[s:e] - g0).astype(BF16)

    n_chunks = n_pad // 1024
    # chunk-major contiguous layouts: one 4KB+ read per partition per chunk
    xth = xam[:, :, 0:H].transpose(0, 2, 1)  # [cores, H, n_pad]
    xt2 = np.ascontiguousarray(
        xth.reshape(N_CORES, 2, 128, n_chunks, 1024)
        .transpose(0, 3, 2, 1, 4)
        .reshape(N_CORES, n_chunks, 128, 2 * 1024)
    )
    xam2 = np.ascontiguousarray(
        xam.reshape(N_CORES, n_chunks, 8, 128, ROW)
        .transpose(0, 1, 3, 2, 4)
        .reshape(N_CORES, n_chunks, 128, 8 * ROW)
    )

    # w1 blocks: [:, ki*256 + 0:256] = W1[ki*128:(ki+1)*128, :]
    w1h = np.zeros((128, 512), dtype=BF16)
    w1h[:, 0:256] = W1[0:128, :].astype(BF16)
    w1h[:, 256:512] = W1[128:256, :].astype(BF16)
    # w2 chunks: [:, ko*HEADS : +HEADS] = W2[ko*128:(ko+1)*128, :]
    w2h = np.zeros((128, 2 * HEADS), dtype=BF16)
    for ko in range(2):
        w2h[:, ko * HEADS : (ko + 1) * HEADS] = W2[
            ko * 128 : (ko + 1) * 128, :
        ].astype(BF16)
    b1h = np.stack([b1[0:128], b1[128:256]], axis=1).astype(np.float32)  # [128, 2]
    # head-mean matrix: rows p=(o%4)*32+jj*4+h -> graph column p//4, value 1/4
    shsh = np.zeros((128, 32), dtype=BF16)
    shsh[np.arange(128), np.arange(128) // HEADS] = BF16(0.25)
    # iota over oct slots, one value per selector column, bcast to all partitions
    ioth = np.broadcast_to(
        (np.arange(SELW) // HEADS).astype(BF16)[None, :], (128, SELW)
    ).copy()

    return T, n_grps, xam2, xt2, w1h, w2h, b1h, shsh, ioth


def kernel(x, batch, W1, b1, W2, num_graphs):
    global LAST_RESULT
    from concourse.bass_utils import run_bass_kernel_spmd

    x = np.asarray(x, dtype=np.float32)
    batch = np.asarray(batch).astype(np.int64)
    W1 = np.asarray(W1, dtype=np.float32)
    b1 = np.asarray(b1, dtype=np.float32)
    W2 = np.asarray(W2, dtype=np.float32)
    G = int(num_graphs)

    T, n_grps, xam, xth, w1h, w2h, b1h, shsh, ioth = _host_prep(
        x, batch, W1, b1, W2, G
    )

    key = (T, n_grps)
    if key not in _NC_CACHE:
        _NC_CACHE[key] = _build_nc(T, n_grps)
    nc = _NC_CACHE[key]

    in_maps = [
        {
            "xam": xam[c],
            "xt": xth[c],
            "w1": w1h,
            "w2": w2h,
            "b1d": b1h,
            "shs": shsh,
            "iot": ioth,
        }
        for c in range(N_CORES)
    ]

    res = run_bass_kernel_spmd(nc, in_maps, core_ids=list(range(N_CORES)))
    LAST_RESULT = res
    return np.concatenate([res.results[c]["out"] for c in range(N_CORES)], axis=0)
